# revision 54
# baseline (speedup 1.0000x reference)
"""MaxPoolAggregator GNN kernel for 8 Trainium2 NeuronCores.

Reference computation:
    H = relu(X[trg] @ fc_w + fc_b)  per edge           [E, 512]
    agg = clamp0(segment_max(H, src))                  [N, 512]
    out = concat([X, agg], 1) @ weights_matrix         [N, 128]

Strategy (src-partitioned, no cross-core traffic):
  - Each core owns a contiguous range of 6272 src nodes and all their edges.
  - Two gather phases with OVERLAPPING int16 index bases (xb is padded with
    256 per-core duplicate rows at each end: base0 = rows [0,32768), base1 =
    rows [17744,50512)), so every node can pad an empty phase-group with a
    real neighbor and the merge is a plain max(a0, a1) — no -inf knockout.
  - Per-node phase degrees are rebalanced over the movable overlap targets,
    then bucketed per-phase (pow2 buckets); nodes sort by the (g0, g1)
    class pair so both phase streams have uniform-g runs sharing one column
    order. Per-class counts are maxed across cores (SPMD-identical program).
  - dma_gather is HW-capped at 512 idxs: one gather per PSUM bank, 8 banks
    share one batched idx DMA; a 10-deep shared rhs ring bounds outstanding
    SWDGE descriptors (more in flight corrupts gathers nondeterministically).
  - Per 512-slot bank: 4 matmuls vs fc_w chunks into one 4-bank PSUM tile;
    chunk 0 drains via DVE tensor_reduce straight from PSUM, chunks 1-3 via
    one ACT f32->bf16 copy into plane b of a shared [128,4,4,512] staging
    tile; one DVE bf16 max-tree (2x mode) drains 4 staged banks at once.
    Tree ops are queued and pumped 1-2 per bank behind their copies so the
    in-order DVE queue never waits on an in-flight ACT copy.
  - max commutes with relu and +bias is per-feature, so bias+relu runs once
    per node on ACT, merged+emitted per 1024-column block as soon as both
    phases' drains for the block are in the stream (overlaps phase 1).
  - Final: out = [X^T; agg^T]^T @ wout as 5 accumulated K=128 matmuls per
    128-node chunk, staged 8 chunks per DRAM store.
Cost-model sim: 544 us/core (baseline pow2-subtile version: 861 us).
"""
import sys

sys.path.insert(0, "/opt/trn_rl_repo")

import numpy as np
import ml_dtypes

N_NODES = 50000
N_EDGES = 800000
D_IN = 128
D_HID = 512
D_OUT = 128
NCORES = 8
NPC = 6272
PAD = 256               # per-core duplicate rows at each end of xb
XB_ROWS = N_NODES + 2 * PAD  # [pad0 (256); X (50000); pad1 (256)]
LO_MAX = 32768 - PAD    # = 32512: phase-0 covers targets < this
HI_MIN = N_NODES + PAD - 32768  # = 17488: phase-1 covers targets >= this
BANK = 512              # slots per PSUM bank
GATHER = 4096           # slots per dma_gather
BUCKETS = [1, 2, 4, 8, 16, 32, 64, 128, 256, 512]
TREE_GROUP = 4          # banks batched into one staged max-tree

_compiled = {}
_host_cache = {}
LAST_RESULTS = None


def _wrap_idx(stream):
    """Wrap a flat int16 index stream [S] into the [128, S//16] gather layout."""
    S = len(stream)
    assert S % 16 == 0
    w = np.zeros((128, S // 16), np.int16)
    s = stream.reshape(-1, 16).T.astype(np.int16)   # [16, S//16]
    for rep in range(8):
        w[rep * 16:(rep + 1) * 16, :] = s
    return w


def _build_host_structures(adjacency):
    src = np.asarray(adjacency[0], dtype=np.int64)
    trg = np.asarray(adjacency[1], dtype=np.int64)
    order = np.argsort(src, kind="stable")
    src_s = src[order]
    trg_s = trg[order]
    deg = np.bincount(src, minlength=N_NODES).astype(np.int64)
    rowptr = np.zeros(N_NODES + 1, np.int64)
    np.cumsum(deg, out=rowptr[1:])
    assert deg.max() <= 512

    # per-node target lists sorted so phase-0 candidates (small t) come first
    key = src_s * (2 * N_NODES) + trg_s
    o2 = np.argsort(key, kind="stable")
    trg_sorted = trg_s[o2]  # within each node: targets ascending

    c_lo = (trg_sorted < HI_MIN).astype(np.int64)
    c_hi = (trg_sorted >= LO_MAX).astype(np.int64)
    cum_lo = np.zeros(N_EDGES + 1, np.int64); np.cumsum(c_lo, out=cum_lo[1:])
    cum_hi = np.zeros(N_EDGES + 1, np.int64); np.cumsum(c_hi, out=cum_hi[1:])
    lo_only = cum_lo[rowptr[1:]] - cum_lo[rowptr[:-1]]
    hi_only = cum_hi[rowptr[1:]] - cum_hi[rowptr[:-1]]
    mid = deg - lo_only - hi_only

    # rebalance movable (overlap) targets: minimize bucket(d0)+bucket(d1)
    B = np.asarray(BUCKETS)

    def buck(v):
        return B[np.searchsorted(B, np.maximum(v, 1))]

    best_cost = None
    best_m = None
    for frac in range(17):
        m = (mid * frac) // 16
        cost = buck(lo_only + m) + buck(hi_only + mid - m)
        if best_cost is None:
            best_cost, best_m = cost.copy(), m.copy()
        else:
            upd = cost < best_cost
            best_cost[upd] = cost[upd]
            best_m[upd] = m[upd]
    d0 = lo_only + best_m          # first d0 targets (ascending) -> phase 0
    d1 = deg - d0
    g0 = buck(d0)
    g1 = buck(d1)

    # class table: cross-core max count per (g0,g1), lexicographic order
    cls_key = g0 * 1000 + g1
    uniq, inv = np.unique(cls_key, return_inverse=True)
    counts = np.zeros((NCORES, len(uniq)), np.int64)
    for k in range(NCORES):
        counts[k] = np.bincount(inv[k * NPC:(k + 1) * NPC], minlength=len(uniq))
    cls_max = counts.max(axis=0)
    cls_g0 = (uniq // 1000).astype(np.int64)
    cls_g1 = (uniq % 1000).astype(np.int64)

    cap = int(cls_max.sum())
    capP = ((cap + 127) // 128) * 128

    # per-phase stream/window tables (identical across cores)
    def build_phase_tables(cls_g):
        # windows[bank] = list of (psum_slot_off, ng, g, col_off); class runs
        # split at bank boundaries (a remainder too small for a group pads to
        # the next bank, slots left pointing at row 0 and covered by no window)
        from collections import defaultdict
        windows = defaultdict(list)
        slot = 0
        col = 0
        for ci in range(len(cls_max)):
            g = int(cls_g[ci])
            left = int(cls_max[ci])
            while left > 0:
                bank = slot // BANK
                off = slot % BANK
                fit = min(left, (BANK - off) // g)
                if fit == 0:
                    slot = (bank + 1) * BANK
                    continue
                windows[bank].append((off, fit, g, col))
                slot += fit * g
                col += fit
                left -= fit
        S_pad = ((slot + GATHER - 1) // GATHER) * GATHER
        n_bank = S_pad // BANK
        wlist = []
        for b in range(n_bank):
            bw = windows.get(b, [])
            # coalesce col-contiguous same-g windows (adjacent classes with
            # equal bucket in this phase)
            merged = []
            for w in bw:
                if merged and merged[-1][2] == w[2] \
                        and merged[-1][0] + merged[-1][1] * merged[-1][2] == w[0] \
                        and merged[-1][3] + merged[-1][1] == w[3]:
                    off, ng, g, col = merged[-1]
                    merged[-1] = (off, ng + w[1], g, col)
                else:
                    merged.append(w)
            wlist.append(merged)
        return S_pad, n_bank, wlist

    S0_pad, n_bank0, win0 = build_phase_tables(cls_g0)
    S1_pad, n_bank1, win1 = build_phase_tables(cls_g1)

    # per-core data: column->node map, slot index streams, pad rows, xt
    # gather index spaces (32768 rows each):
    #   phase 0: xb rows [0, 32768)      -> idx = t + PAD, pad0 idx = r
    #   phase 1: xb rows [17744, 50512)  -> idx = t - HI_MIN, pad1 idx = 32512 + r
    cores = []
    for k in range(NCORES):
        nodes = np.arange(k * NPC, min((k + 1) * NPC, N_NODES))
        node_cls = inv[k * NPC:(k + 1) * NPC]
        col_node = np.full(cap, -1, np.int64)
        streams = {0: np.zeros(S0_pad, np.int64), 1: np.zeros(S1_pad, np.int64)}
        pad_rows = {0: [], 1: []}
        col_base = 0
        for ci in range(len(cls_max)):
            sel = nodes[node_cls == ci]
            col_node[col_base:col_base + len(sel)] = sel
            col_base += int(cls_max[ci])
        for ph, wins in ((0, win0), (1, win1)):
            st = streams[ph]
            for bank_idx, bw in enumerate(wins):
                for (off, ng, g, col) in bw:
                    base = bank_idx * BANK + off
                    for i in range(ng):
                        n = col_node[col + i]
                        if n < 0:
                            continue  # dummy: stays 0
                        n = int(n)
                        lo0, hi0 = rowptr[n], rowptr[n + 1]
                        tt = trg_sorted[lo0:hi0]
                        dd0 = int(d0[n])
                        tp = tt[:dd0] if ph == 0 else tt[dd0:]
                        if len(tp):
                            arr = np.empty(g, np.int64)
                            arr[:len(tp)] = (tp + PAD) if ph == 0 else (tp - HI_MIN)
                            arr[len(tp):] = arr[0]
                        else:
                            # pad from the other phase's targets if in range,
                            # else via a per-core duplicate row
                            alt = tt[dd0:] if ph == 0 else tt[:dd0]
                            ok = alt[alt < LO_MAX] if ph == 0 else alt[alt >= HI_MIN]
                            if len(ok):
                                v = int(ok[0]) + PAD if ph == 0 else int(ok[0]) - HI_MIN
                            else:
                                r = len(pad_rows[ph])
                                assert r < PAD, "pad row overflow"
                                pad_rows[ph].append(int(tt[0]))
                                v = r if ph == 0 else (LO_MAX + r)
                            arr = np.full(g, v, np.int64)
                        assert arr.min() >= 0 and arr.max() < 32768
                        st[base + i * g: base + (i + 1) * g] = arr
        gidx0 = _wrap_idx(streams[0])
        gidx1 = _wrap_idx(streams[1])
        cores.append(dict(col_node=col_node, gidx0=gidx0, gidx1=gidx1,
                          pad0=np.asarray(pad_rows[0], np.int64),
                          pad1=np.asarray(pad_rows[1], np.int64)))

    return dict(cores=cores, cap=cap, capP=capP,
                S0_pad=S0_pad, S1_pad=S1_pad,
                n_bank0=n_bank0, n_bank1=n_bank1, win0=win0, win1=win1)


def _tree_ops(nc, mybir, v, out_ap, g):
    """Thunks that max-reduce the last axis of v [128, ..., g] bf16 into
    out_ap (in-place tree). Emitted one-per-bank to avoid DVE queue bursts."""
    MAX = mybir.AluOpType.max
    ops = []

    def tt(out, in0, in1):
        ops.append(lambda: nc.vector.tensor_tensor(out=out, in0=in0, in1=in1,
                                                   op=MAX))

    if g == 1:
        tt(out_ap, v[..., 0], v[..., 0])
        return ops
    w = g
    p2 = 1
    while p2 * 2 <= w:
        p2 *= 2
    if p2 != w:
        rem = w - p2
        tt(v[..., :rem], v[..., :rem], v[..., p2:w])
        w = p2
    while w > 2:
        h = w // 2
        tt(v[..., :h], v[..., :h], v[..., h:w])
        w = h
    if w == 2:
        tt(out_ap, v[..., 0], v[..., 1])
    else:
        tt(out_ap, v[..., 0], v[..., 0])
    return ops


def _tree_reduce(nc, mybir, v, out_ap, ng_total, g):
    for op in _tree_ops(nc, mybir, v, out_ap, g):
        op()


def _build_program(host, stage="full"):
    import concourse.bass as bass
    import concourse.bacc as bacc
    import concourse.mybir as mybir
    import concourse.tile as tile
    from concourse import library_config

    bf16 = mybir.dt.bfloat16
    f32 = mybir.dt.float32
    i16 = mybir.dt.int16
    AX = mybir.AxisListType.X
    MAX = mybir.AluOpType.max

    capP = host["capP"]
    S0_pad, S1_pad = host["S0_pad"], host["S1_pad"]
    n_bank = {0: host["n_bank0"], 1: host["n_bank1"]}
    wins = {0: host["win0"], 1: host["win1"]}

    nc = bacc.Bacc(None, target_bir_lowering=False, num_swdge_queues=4)
    xb = nc.dram_tensor("xb", [XB_ROWS, D_IN], bf16, kind="ExternalInput")
    gidx0 = nc.dram_tensor("gidx0", [128, S0_pad // 16], i16, kind="ExternalInput")
    gidx1 = nc.dram_tensor("gidx1", [128, S1_pad // 16], i16, kind="ExternalInput")
    wfc = nc.dram_tensor("wfc", [D_IN, D_HID], bf16, kind="ExternalInput")
    fcb = nc.dram_tensor("fcb", [128, 4], f32, kind="ExternalInput")
    xt = nc.dram_tensor("xt", [128, capP], bf16, kind="ExternalInput")
    wout = nc.dram_tensor("wout", [128, 5 * D_OUT], bf16, kind="ExternalInput")
    outp = nc.dram_tensor("out", [capP, D_OUT], f32, kind="ExternalOutput")
    gidx = {0: gidx0, 1: gidx1}

    with tile.TileContext(nc) as tc:
        with tc.tile_pool(name="const", bufs=1) as cpool, \
             tc.tile_pool(name="io", bufs=3) as iopool, \
             tc.tile_pool(name="work", bufs=4) as wpool, \
             tc.tile_pool(name="ost", bufs=2) as ostpool:
            nc.gpsimd.load_library(library_config.mlp)
            wfc_sb = cpool.tile([128, D_HID], bf16, name="wfc_sb")
            nc.sync.dma_start(wfc_sb[:], wfc[:])
            fcb_sb = cpool.tile([128, 4], f32, name="fcb_sb")
            nc.sync.dma_start(fcb_sb[:], fcb[:])
            agg = [cpool.tile([128, 4, capP], bf16, name=f"agg{p}") for p in range(2)]
            cap = host["cap"]
            if capP > cap:
                for p in range(2):
                    nc.vector.memset(agg[p][:, :, cap:], 0.0)
            xt_sb = cpool.tile([128, capP], bf16, name="xt_sb")
            nc.sync.dma_start(xt_sb[:], xt[:])
            wout_sb = cpool.tile([128, 5 * D_OUT], bf16, name="wout_sb")
            nc.sync.dma_start(wout_sb[:], wout[:])

            # merge + bias/relu + final matmul, emitted per column block as
            # soon as both phases' drains for the block are in the stream.
            # Ops are queued and pumped a few per bank to avoid bunching.
            MBLK = 1024
            fin_q = []

            def merge_blocks(upto):
                m0 = merge_blocks.done
                while m0 + MBLK <= upto or (upto >= capP and m0 < capP):
                    end = min(m0 + MBLK, capP)
                    for h in range(4):
                        def mop(h=h, m0=m0, end=end):
                            nc.vector.tensor_tensor(
                                out=agg[0][:, h, m0:end],
                                in0=agg[0][:, h, m0:end],
                                in1=agg[1][:, h, m0:end], op=MAX)
                            nc.scalar.activation(
                                out=agg[0][:, h, m0:end],
                                in_=agg[0][:, h, m0:end],
                                func=mybir.ActivationFunctionType.Relu,
                                bias=fcb_sb[:, h:h + 1], scale=1.0)
                        fin_q.append(mop)
                    m0 = end
                merge_blocks.done = m0
            merge_blocks.done = 0

            def fin_pump(n=2):
                while n and fin_q:
                    fin_q.pop(0)()
                    n -= 1

            # after phase-1 bank b, columns below the first col of bank b+1
            # are fully drained (cols are monotonic across banks)
            bank1_next_col = []
            for b in range(n_bank[1]):
                nxt = cap
                for b2 in range(b + 1, n_bank[1]):
                    if wins[1][b2]:
                        nxt = wins[1][b2][0][3]
                        break
                bank1_next_col.append(nxt if nxt < cap else capP)

            with tc.tile_pool(name="mm", bufs=2, space="PSUM") as mmpool:
                drain_rot = 0
                # Staged drains: each bank's non-direct chunks are ACT-copied
                # into plane b of a shared [128, 4, TREE_GROUP, BANK] tile;
                # one batched DVE tree drains the whole group (init overhead
                # amortized 4x). Groups span consecutive single-full-window
                # banks of equal g with contiguous columns. Tree ops are
                # queued and pumped 1-2 per bank so the in-order DVE queue
                # never bursts or waits on an in-flight copy.
                open_group = None   # (st, n_dir, ph, g, col0, ng, nbanks, last_bank)
                tree_q = []         # (thunk | None, unlock_col | None)
                state = {"unlocked": 0}

                def close_group():
                    nonlocal open_group
                    if open_group is None:
                        return
                    st_p, n_dir_p, ph_p, g, col0, ng, nb_, last_b = open_group
                    open_group = None
                    ops = []
                    if n_dir_p < 4:
                        v = st_p[:, n_dir_p:, :nb_, :] \
                            .rearrange("p k b (n g) -> p k (b n) g", g=g)
                        ops = _tree_ops(
                            nc, mybir, v,
                            agg[ph_p][:, n_dir_p:, col0:col0 + nb_ * ng], g)
                    unlock = bank1_next_col[last_b] if ph_p == 1 else None
                    if ops:
                        for i, op in enumerate(ops):
                            tree_q.append(
                                (op, unlock if i == len(ops) - 1 else None))
                    else:
                        tree_q.append((None, unlock))

                def pump(n=2):
                    while n and tree_q:
                        op, unlock = tree_q.pop(0)
                        if op is not None:
                            op()
                        if unlock is not None:
                            state["unlocked"] = unlock
                        n -= 1

                for ph in range(2):
                    base = xb[:32768, :] if ph == 0 else xb[XB_ROWS - 32768:, :]
                    n_gather = (n_bank[ph] * BANK) // GATHER
                    for gb in range(n_gather):
                        idx_sb = iopool.tile([128, GATHER // 16], i16, tag="idx",
                                             name="idx_sb")
                        nc.sync.dma_start(
                            idx_sb[:],
                            gidx[ph][:, gb * (GATHER // 16):(gb + 1) * (GATHER // 16)])
                        # HW dma_gather caps at 512 idxs: one gather+tile per
                        # bank, sharing one batched idx load per 4096 slots.
                        # Single shared ring tag bounds outstanding gathers
                        # (SWDGE descriptor ring holds ~1024 descs).
                        rhs_tiles = []
                        for bb in range(GATHER // BANK):
                            rhs = iopool.tile([128, 1, BANK], bf16, tag="rhs",
                                              name="rhs", bufs=10)
                            nc.gpsimd.dma_gather(
                                out_ap=rhs[:],
                                in_ap=base,
                                idxs_ap=idx_sb[:, bb * (BANK // 16):(bb + 1) * (BANK // 16)],
                                num_idxs=BANK, num_idxs_reg=BANK,
                                elem_size=D_IN, transpose=True,
                                queue_num=(gb * (GATHER // BANK) + bb) % 4)
                            rhs_tiles.append(rhs)
                        if stage == "gather":
                            nc.vector.tensor_tensor(
                                out=agg[ph][:, 0, :256], in0=rhs_tiles[0][:, 0, :256],
                                in1=rhs_tiles[0][:, 0, 256:512], op=MAX)
                            continue
                        for bb in range(GATHER // BANK):
                            bank_idx = gb * (GATHER // BANK) + bb
                            bank_wins = wins[ph][bank_idx]
                            if not bank_wins:
                                continue
                            pm = mmpool.tile([128, 4, BANK], f32, tag="mm",
                                             name="pm")
                            rslice = rhs_tiles[bb][:, 0, :]
                            for h in range(4):
                                nc.tensor.matmul(
                                    out=pm[:, h, :],
                                    lhsT=wfc_sb[:, h * 128:(h + 1) * 128],
                                    rhs=rslice, start=True, stop=True)
                            # drain: n_dir chunks via DVE tensor_reduce from
                            # PSUM, rest via ACT copy + DVE bf16 tree
                            if stage == "direct":
                                for h in range(4):
                                    for (off, ng, g, col) in bank_wins:
                                        nc.vector.tensor_reduce(
                                            out=agg[ph][:, h, col:col + ng],
                                            in_=pm[:, h, off:off + ng * g]
                                                .rearrange("p (n g) -> p n g", g=g),
                                            axis=AX, op=MAX)
                                continue
                            # groupable: one full-bank window starting at 0
                            w0 = bank_wins[0]
                            groupable = (len(bank_wins) == 1 and w0[0] == 0
                                         and w0[1] * w0[2] == BANK)
                            joined = False
                            if groupable and open_group is not None:
                                st_o, nd_o, ph_o, g_o, col0_o, ng_o, nb_o, _ = \
                                    open_group
                                if (ph_o == ph and g_o == w0[2]
                                        and col0_o + nb_o * ng_o == w0[3]
                                        and nb_o < TREE_GROUP):
                                    joined = True
                            if not joined:
                                close_group()
                            if open_group is None:
                                n_dir = 1
                                drain_rot += 1
                                st = wpool.tile([128, 4, TREE_GROUP, BANK],
                                                bf16, tag="st", name="st")
                                if groupable:
                                    open_group = (st, n_dir, ph, w0[2], w0[3],
                                                  w0[1], 0, bank_idx)
                            else:
                                st = open_group[0]
                                n_dir = open_group[1]

                            for h in range(n_dir):
                                for (off, ng, g, col) in bank_wins:
                                    nc.vector.tensor_reduce(
                                        out=agg[ph][:, h, col:col + ng],
                                        in_=pm[:, h, off:off + ng * g]
                                            .rearrange("p (n g) -> p n g", g=g),
                                        axis=AX, op=MAX)

                            if groupable:
                                plane = open_group[6]
                                if n_dir < 4:
                                    nc.scalar.copy(
                                        out=st[:, n_dir:, plane, :],
                                        in_=pm[:, n_dir:, :])
                                open_group = (st, n_dir, ph, open_group[3],
                                              open_group[4], open_group[5],
                                              plane + 1, bank_idx)
                                if open_group[6] == TREE_GROUP:
                                    close_group()
                                pump(2)
                            else:
                                # irregular bank: per-window trees, queued
                                if n_dir < 4:
                                    nc.scalar.copy(out=st[:, n_dir:, 0, :],
                                                   in_=pm[:, n_dir:, :])
                                    ops = []
                                    for (off, ng, g, col) in bank_wins:
                                        v = st[:, n_dir:, 0, off:off + ng * g] \
                                            .rearrange("p k (n g) -> p k n g",
                                                       g=g)
                                        ops += _tree_ops(
                                            nc, mybir, v,
                                            agg[ph][:, n_dir:, col:col + ng],
                                            g)
                                    for op in ops:
                                        op()
                                unlock = (bank1_next_col[bank_idx]
                                          if ph == 1 else None)
                                tree_q.append((None, unlock))
                            if ph == 1 and stage == "full":
                                merge_blocks(state["unlocked"])
                                fin_pump(2)

                    close_group()
                    pump(len(tree_q))
                merge_blocks(capP)
                fin_pump(len(fin_q))

            # final data-parallel matmul over node chunks
            with tc.tile_pool(name="fin", bufs=4, space="PSUM") as finpool:
                n_chunk = capP // 128
                mb = 0
                while mb < n_chunk:
                    nb = min(8, n_chunk - mb)
                    osb = ostpool.tile([128, nb, D_OUT], f32, tag="osb",
                                       name="osb")
                    for mi in range(nb):
                        m = mb + mi
                        pm2 = finpool.tile([128, D_OUT], f32, tag="fmm",
                                           name="pm2")
                        for c in range(5):
                            lhsT = (xt_sb[:, m * 128:(m + 1) * 128] if c == 0
                                    else agg[0][:, c - 1,
                                                m * 128:(m + 1) * 128])
                            nc.tensor.matmul(
                                out=pm2[:], lhsT=lhsT,
                                rhs=wout_sb[:, c * D_OUT:(c + 1) * D_OUT],
                                start=(c == 0), stop=(c == 4))
                        nc.scalar.copy(out=osb[:, mi, :], in_=pm2[:])
                    nc.sync.dma_start(
                        outp[mb * 128:(mb + nb) * 128, :]
                            .rearrange("(c n) f -> n c f", c=nb),
                        osb[:])
                    mb += nb

    nc.finalize()
    return nc


def kernel(input_matrix, fc_w, fc_b, weights_matrix, adjacency_coo_matrix):
    global LAST_RESULTS
    from concourse.bass_utils import run_bass_kernel_spmd

    X = np.asarray(input_matrix, np.float32)
    Wfc = np.asarray(fc_w, np.float32)
    bfc = np.asarray(fc_b, np.float32)
    Wout = np.asarray(weights_matrix, np.float32)

    adj = np.asarray(adjacency_coo_matrix)
    hkey = hash(adj[:, ::1024].tobytes()) ^ hash(adj[:, -7:].tobytes())
    host = _host_cache.get(hkey)
    if host is None:
        host = _build_host_structures(adj)
        _host_cache[hkey] = host

    nc = _compiled.get(hkey)
    if nc is None:
        nc = _build_program(host)
        _compiled[hkey] = nc

    capP = host["capP"]
    Xb = X.astype(ml_dtypes.bfloat16)
    wfc_in = Wfc.astype(ml_dtypes.bfloat16)
    fcb_in = np.ascontiguousarray(bfc.reshape(4, 128).T.astype(np.float32))
    wout_in = np.ascontiguousarray(
        Wout.reshape(5, 128, D_OUT).transpose(1, 0, 2).reshape(128, 5 * D_OUT)
    ).astype(ml_dtypes.bfloat16)

    in_maps = []
    for k in range(NCORES):
        hc = host["cores"][k]
        col_node = hc["col_node"]
        xt_in = np.zeros((capP, D_IN), np.float32)
        valid = col_node >= 0
        xt_in[:len(col_node)][valid] = X[col_node[valid]]
        xt_in = np.ascontiguousarray(xt_in.T).astype(ml_dtypes.bfloat16)
        xb_core = np.zeros((XB_ROWS, D_IN), ml_dtypes.bfloat16)
        xb_core[PAD:PAD + N_NODES] = Xb
        if len(hc["pad0"]):
            xb_core[:len(hc["pad0"])] = Xb[hc["pad0"]]
        if len(hc["pad1"]):
            xb_core[PAD + N_NODES:PAD + N_NODES + len(hc["pad1"])] = Xb[hc["pad1"]]
        in_maps.append({
            "xb": xb_core,
            "gidx0": hc["gidx0"],
            "gidx1": hc["gidx1"],
            "wfc": wfc_in,
            "fcb": fcb_in,
            "xt": xt_in,
            "wout": wout_in,
        })

    res = run_bass_kernel_spmd(nc, in_maps, list(range(NCORES)))
    LAST_RESULTS = res

    out_full = np.zeros((N_NODES, D_OUT), np.float32)
    for k in range(NCORES):
        got = np.asarray(res.results[k]["out"], np.float32)
        col_node = host["cores"][k]["col_node"]
        valid = col_node >= 0
        out_full[col_node[valid]] = got[:len(col_node)][valid]
    return out_full


# revision 59
# speedup vs baseline: 1.0114x; 1.0114x over previous
"""MaxPoolAggregator GNN kernel for 8 Trainium2 NeuronCores.

Reference computation:
    H = relu(X[trg] @ fc_w + fc_b)  per edge           [E, 512]
    agg = clamp0(segment_max(H, src))                  [N, 512]
    out = concat([X, agg], 1) @ weights_matrix         [N, 128]

Strategy (src-partitioned, no cross-core traffic):
  - Each core owns a contiguous range of 6272 src nodes and all their edges.
  - Two gather phases with OVERLAPPING int16 index bases (xb is padded with
    256 per-core duplicate rows at each end: base0 = rows [0,32768), base1 =
    rows [17744,50512)), so every node can pad an empty phase-group with a
    real neighbor and the merge is a plain max(a0, a1) — no -inf knockout.
  - Per-node phase degrees are rebalanced over the movable overlap targets,
    then bucketed per-phase (pow2 buckets); nodes sort by the (g0, g1)
    class pair so both phase streams have uniform-g runs sharing one column
    order. Per-class counts are maxed across cores (SPMD-identical program).
  - dma_gather is HW-capped at 512 idxs: one gather per PSUM bank, 8 banks
    share one batched idx DMA; a 10-deep shared rhs ring bounds outstanding
    SWDGE descriptors (more in flight corrupts gathers nondeterministically).
  - Per 512-slot bank: 4 matmuls vs fc_w chunks into one 4-bank PSUM tile;
    chunk 0 drains via DVE tensor_reduce straight from PSUM, chunks 1-3 via
    one ACT f32->bf16 copy into plane b of a shared [128,4,4,512] staging
    tile; one DVE bf16 max-tree (2x mode) drains 4 staged banks at once.
    Tree ops are queued and pumped 1-2 per bank behind their copies so the
    in-order DVE queue never waits on an in-flight ACT copy.
  - max commutes with relu and +bias is per-feature, so bias+relu runs once
    per node on ACT, merged+emitted per 1024-column block as soon as both
    phases' drains for the block are in the stream (overlaps phase 1).
  - Final: out = [X^T; agg^T]^T @ wout as 5 accumulated K=128 matmuls per
    128-node chunk, staged 8 chunks per DRAM store.
Cost-model sim: 544 us/core (baseline pow2-subtile version: 861 us).
"""
import sys

sys.path.insert(0, "/opt/trn_rl_repo")

import numpy as np
import ml_dtypes

N_NODES = 50000
N_EDGES = 800000
D_IN = 128
D_HID = 512
D_OUT = 128
NCORES = 8
NPC = 6272
PAD = 256               # per-core duplicate rows at each end of xb
XB_ROWS = N_NODES + 2 * PAD  # [pad0 (256); X (50000); pad1 (256)]
LO_MAX = 32768 - PAD    # = 32512: phase-0 covers targets < this
HI_MIN = N_NODES + PAD - 32768  # = 17488: phase-1 covers targets >= this
BANK = 512              # slots per PSUM bank
GATHER = 4096           # slots per dma_gather
BUCKETS = [1, 2, 4, 8, 16, 32, 64, 128, 256, 512]
TREE_GROUP = 4          # banks batched into one staged max-tree

_compiled = {}
_host_cache = {}
LAST_RESULTS = None


def _wrap_idx(stream):
    """Wrap a flat int16 index stream [S] into the [128, S//16] gather layout."""
    S = len(stream)
    assert S % 16 == 0
    w = np.zeros((128, S // 16), np.int16)
    s = stream.reshape(-1, 16).T.astype(np.int16)   # [16, S//16]
    for rep in range(8):
        w[rep * 16:(rep + 1) * 16, :] = s
    return w


def _build_host_structures(adjacency):
    src = np.asarray(adjacency[0], dtype=np.int64)
    trg = np.asarray(adjacency[1], dtype=np.int64)
    order = np.argsort(src, kind="stable")
    src_s = src[order]
    trg_s = trg[order]
    deg = np.bincount(src, minlength=N_NODES).astype(np.int64)
    rowptr = np.zeros(N_NODES + 1, np.int64)
    np.cumsum(deg, out=rowptr[1:])
    assert deg.max() <= 512

    # per-node target lists sorted so phase-0 candidates (small t) come first
    key = src_s * (2 * N_NODES) + trg_s
    o2 = np.argsort(key, kind="stable")
    trg_sorted = trg_s[o2]  # within each node: targets ascending

    c_lo = (trg_sorted < HI_MIN).astype(np.int64)
    c_hi = (trg_sorted >= LO_MAX).astype(np.int64)
    cum_lo = np.zeros(N_EDGES + 1, np.int64); np.cumsum(c_lo, out=cum_lo[1:])
    cum_hi = np.zeros(N_EDGES + 1, np.int64); np.cumsum(c_hi, out=cum_hi[1:])
    lo_only = cum_lo[rowptr[1:]] - cum_lo[rowptr[:-1]]
    hi_only = cum_hi[rowptr[1:]] - cum_hi[rowptr[:-1]]
    mid = deg - lo_only - hi_only

    # rebalance movable (overlap) targets: minimize bucket(d0)+bucket(d1)
    B = np.asarray(BUCKETS)

    def buck(v):
        return B[np.searchsorted(B, np.maximum(v, 1))]

    best_cost = None
    best_m = None
    for frac in range(17):
        m = (mid * frac) // 16
        cost = buck(lo_only + m) + buck(hi_only + mid - m)
        if best_cost is None:
            best_cost, best_m = cost.copy(), m.copy()
        else:
            upd = cost < best_cost
            best_cost[upd] = cost[upd]
            best_m[upd] = m[upd]
    d0 = lo_only + best_m          # first d0 targets (ascending) -> phase 0
    d1 = deg - d0
    g0 = buck(d0)
    g1 = buck(d1)

    # class table: cross-core max count per (g0,g1), lexicographic order
    cls_key = g0 * 1000 + g1
    uniq, inv = np.unique(cls_key, return_inverse=True)
    counts = np.zeros((NCORES, len(uniq)), np.int64)
    for k in range(NCORES):
        counts[k] = np.bincount(inv[k * NPC:(k + 1) * NPC], minlength=len(uniq))
    cls_max = counts.max(axis=0)
    cls_g0 = (uniq // 1000).astype(np.int64)
    cls_g1 = (uniq % 1000).astype(np.int64)

    cap = int(cls_max.sum())
    capP = ((cap + 127) // 128) * 128

    # per-phase stream/window tables (identical across cores)
    def build_phase_tables(cls_g):
        # windows[bank] = list of (psum_slot_off, ng, g, col_off); class runs
        # split at bank boundaries (a remainder too small for a group pads to
        # the next bank, slots left pointing at row 0 and covered by no window)
        from collections import defaultdict
        windows = defaultdict(list)
        slot = 0
        col = 0
        for ci in range(len(cls_max)):
            g = int(cls_g[ci])
            left = int(cls_max[ci])
            while left > 0:
                bank = slot // BANK
                off = slot % BANK
                fit = min(left, (BANK - off) // g)
                if fit == 0:
                    slot = (bank + 1) * BANK
                    continue
                windows[bank].append((off, fit, g, col))
                slot += fit * g
                col += fit
                left -= fit
        S_pad = ((slot + GATHER - 1) // GATHER) * GATHER
        n_bank = S_pad // BANK
        wlist = []
        for b in range(n_bank):
            bw = windows.get(b, [])
            # coalesce col-contiguous same-g windows (adjacent classes with
            # equal bucket in this phase)
            merged = []
            for w in bw:
                if merged and merged[-1][2] == w[2] \
                        and merged[-1][0] + merged[-1][1] * merged[-1][2] == w[0] \
                        and merged[-1][3] + merged[-1][1] == w[3]:
                    off, ng, g, col = merged[-1]
                    merged[-1] = (off, ng + w[1], g, col)
                else:
                    merged.append(w)
            wlist.append(merged)
        return S_pad, n_bank, wlist

    S0_pad, n_bank0, win0 = build_phase_tables(cls_g0)
    S1_pad, n_bank1, win1 = build_phase_tables(cls_g1)

    # per-core data: column->node map, slot index streams, pad rows, xt
    # gather index spaces (32768 rows each):
    #   phase 0: xb rows [0, 32768)      -> idx = t + PAD, pad0 idx = r
    #   phase 1: xb rows [17744, 50512)  -> idx = t - HI_MIN, pad1 idx = 32512 + r
    cores = []
    for k in range(NCORES):
        nodes = np.arange(k * NPC, min((k + 1) * NPC, N_NODES))
        node_cls = inv[k * NPC:(k + 1) * NPC]
        col_node = np.full(cap, -1, np.int64)
        streams = {0: np.zeros(S0_pad, np.int64), 1: np.zeros(S1_pad, np.int64)}
        pad_rows = {0: [], 1: []}
        col_base = 0
        for ci in range(len(cls_max)):
            sel = nodes[node_cls == ci]
            col_node[col_base:col_base + len(sel)] = sel
            col_base += int(cls_max[ci])
        for ph, wins in ((0, win0), (1, win1)):
            st = streams[ph]
            for bank_idx, bw in enumerate(wins):
                for (off, ng, g, col) in bw:
                    base = bank_idx * BANK + off
                    for i in range(ng):
                        n = col_node[col + i]
                        if n < 0:
                            continue  # dummy: stays 0
                        n = int(n)
                        lo0, hi0 = rowptr[n], rowptr[n + 1]
                        tt = trg_sorted[lo0:hi0]
                        dd0 = int(d0[n])
                        tp = tt[:dd0] if ph == 0 else tt[dd0:]
                        if len(tp):
                            arr = np.empty(g, np.int64)
                            arr[:len(tp)] = (tp + PAD) if ph == 0 else (tp - HI_MIN)
                            arr[len(tp):] = arr[0]
                        else:
                            # pad from the other phase's targets if in range,
                            # else via a per-core duplicate row
                            alt = tt[dd0:] if ph == 0 else tt[:dd0]
                            ok = alt[alt < LO_MAX] if ph == 0 else alt[alt >= HI_MIN]
                            if len(ok):
                                v = int(ok[0]) + PAD if ph == 0 else int(ok[0]) - HI_MIN
                            else:
                                r = len(pad_rows[ph])
                                assert r < PAD, "pad row overflow"
                                pad_rows[ph].append(int(tt[0]))
                                v = r if ph == 0 else (LO_MAX + r)
                            arr = np.full(g, v, np.int64)
                        assert arr.min() >= 0 and arr.max() < 32768
                        st[base + i * g: base + (i + 1) * g] = arr
        gidx0 = _wrap_idx(streams[0])
        gidx1 = _wrap_idx(streams[1])
        cores.append(dict(col_node=col_node, gidx0=gidx0, gidx1=gidx1,
                          pad0=np.asarray(pad_rows[0], np.int64),
                          pad1=np.asarray(pad_rows[1], np.int64)))

    return dict(cores=cores, cap=cap, capP=capP,
                S0_pad=S0_pad, S1_pad=S1_pad,
                n_bank0=n_bank0, n_bank1=n_bank1, win0=win0, win1=win1)


def _tree_ops(nc, mybir, v, out_ap, g):
    """Thunks that max-reduce the last axis of v [128, ..., g] bf16 into
    out_ap (in-place tree). Emitted one-per-bank to avoid DVE queue bursts."""
    MAX = mybir.AluOpType.max
    ops = []

    def tt(out, in0, in1):
        ops.append(lambda: nc.vector.tensor_tensor(out=out, in0=in0, in1=in1,
                                                   op=MAX))

    if g == 1:
        tt(out_ap, v[..., 0], v[..., 0])
        return ops
    w = g
    p2 = 1
    while p2 * 2 <= w:
        p2 *= 2
    if p2 != w:
        rem = w - p2
        tt(v[..., :rem], v[..., :rem], v[..., p2:w])
        w = p2
    while w > 2:
        h = w // 2
        tt(v[..., :h], v[..., :h], v[..., h:w])
        w = h
    if w == 2:
        tt(out_ap, v[..., 0], v[..., 1])
    else:
        tt(out_ap, v[..., 0], v[..., 0])
    return ops


def _tree_reduce(nc, mybir, v, out_ap, ng_total, g):
    for op in _tree_ops(nc, mybir, v, out_ap, g):
        op()


def _build_program(host, stage="full"):
    import concourse.bass as bass
    import concourse.bacc as bacc
    import concourse.mybir as mybir
    import concourse.tile as tile
    from concourse import library_config

    bf16 = mybir.dt.bfloat16
    f32 = mybir.dt.float32
    i16 = mybir.dt.int16
    AX = mybir.AxisListType.X
    MAX = mybir.AluOpType.max

    capP = host["capP"]
    S0_pad, S1_pad = host["S0_pad"], host["S1_pad"]
    n_bank = {0: host["n_bank0"], 1: host["n_bank1"]}
    wins = {0: host["win0"], 1: host["win1"]}

    nc = bacc.Bacc(None, target_bir_lowering=False, num_swdge_queues=4)
    xb = nc.dram_tensor("xb", [XB_ROWS, D_IN], bf16, kind="ExternalInput")
    gidx0 = nc.dram_tensor("gidx0", [128, S0_pad // 16], i16, kind="ExternalInput")
    gidx1 = nc.dram_tensor("gidx1", [128, S1_pad // 16], i16, kind="ExternalInput")
    wfc = nc.dram_tensor("wfc", [D_IN, D_HID], bf16, kind="ExternalInput")
    fcb = nc.dram_tensor("fcb", [128, 4], f32, kind="ExternalInput")
    xt = nc.dram_tensor("xt", [128, capP], bf16, kind="ExternalInput")
    wout = nc.dram_tensor("wout", [128, 5 * D_OUT], bf16, kind="ExternalInput")
    outp = nc.dram_tensor("out", [capP, D_OUT], f32, kind="ExternalOutput")
    gidx = {0: gidx0, 1: gidx1}

    with tile.TileContext(nc) as tc:
        with tc.tile_pool(name="const", bufs=1) as cpool, \
             tc.tile_pool(name="io", bufs=3) as iopool, \
             tc.tile_pool(name="work", bufs=4) as wpool, \
             tc.tile_pool(name="ost", bufs=2) as ostpool:
            nc.gpsimd.load_library(library_config.mlp)
            wfc_sb = cpool.tile([128, D_HID], bf16, name="wfc_sb")
            nc.sync.dma_start(wfc_sb[:], wfc[:])
            fcb_sb = cpool.tile([128, 4], f32, name="fcb_sb")
            nc.sync.dma_start(fcb_sb[:], fcb[:])
            agg = [cpool.tile([128, 4, capP], bf16, name=f"agg{p}") for p in range(2)]
            cap = host["cap"]
            if capP > cap:
                for p in range(2):
                    nc.vector.memset(agg[p][:, :, cap:], 0.0)
            xt_sb = cpool.tile([128, capP], bf16, name="xt_sb")
            nc.sync.dma_start(xt_sb[:], xt[:])
            wout_sb = cpool.tile([128, 5 * D_OUT], bf16, name="wout_sb")
            nc.sync.dma_start(wout_sb[:], wout[:])

            # merge + bias/relu + final matmul, emitted per column block as
            # soon as both phases' drains for the block are in the stream.
            # Ops are queued and pumped a few per bank to avoid bunching.
            MBLK = 512
            fin_q = []

            def merge_blocks(upto):
                m0 = merge_blocks.done
                while m0 + MBLK <= upto or (upto >= capP and m0 < capP):
                    end = min(m0 + MBLK, capP)
                    for h in range(4):
                        def mop(h=h, m0=m0, end=end):
                            nc.vector.tensor_tensor(
                                out=agg[0][:, h, m0:end],
                                in0=agg[0][:, h, m0:end],
                                in1=agg[1][:, h, m0:end], op=MAX)
                            nc.scalar.activation(
                                out=agg[0][:, h, m0:end],
                                in_=agg[0][:, h, m0:end],
                                func=mybir.ActivationFunctionType.Relu,
                                bias=fcb_sb[:, h:h + 1], scale=1.0)
                        fin_q.append(mop)
                    m0 = end
                merge_blocks.done = m0
            merge_blocks.done = 0

            def fin_pump(n=2):
                while n and fin_q:
                    fin_q.pop(0)()
                    n -= 1

            # after phase-1 bank b, columns below the first col of bank b+1
            # are fully drained (cols are monotonic across banks)
            bank1_next_col = []
            for b in range(n_bank[1]):
                nxt = cap
                for b2 in range(b + 1, n_bank[1]):
                    if wins[1][b2]:
                        nxt = wins[1][b2][0][3]
                        break
                bank1_next_col.append(nxt if nxt < cap else capP)

            with tc.tile_pool(name="mm", bufs=2, space="PSUM") as mmpool:
                drain_rot = 0
                # Staged drains: each bank's non-direct chunks are ACT-copied
                # into plane b of a shared [128, 4, TREE_GROUP, BANK] tile;
                # one batched DVE tree drains the whole group (init overhead
                # amortized 4x). Groups span consecutive single-full-window
                # banks of equal g with contiguous columns. Tree ops are
                # queued and pumped 1-2 per bank so the in-order DVE queue
                # never bursts or waits on an in-flight copy.
                open_group = None   # (st, n_dir, ph, g, col0, ng, nbanks, last_bank)
                tree_q = []         # (thunk | None, unlock_col | None)
                state = {"unlocked": 0}

                def close_group():
                    nonlocal open_group
                    if open_group is None:
                        return
                    st_p, n_dir_p, ph_p, g, col0, ng, nb_, last_b = open_group
                    open_group = None
                    ops = []
                    if n_dir_p < 4:
                        v = st_p[:, n_dir_p:, :nb_, :] \
                            .rearrange("p k b (n g) -> p k (b n) g", g=g)
                        ops = _tree_ops(
                            nc, mybir, v,
                            agg[ph_p][:, n_dir_p:, col0:col0 + nb_ * ng], g)
                    unlock = bank1_next_col[last_b] if ph_p == 1 else None
                    if ops:
                        for i, op in enumerate(ops):
                            tree_q.append(
                                (op, unlock if i == len(ops) - 1 else None))
                    else:
                        tree_q.append((None, unlock))

                def pump(n=2):
                    while n and tree_q:
                        op, unlock = tree_q.pop(0)
                        if op is not None:
                            op()
                        if unlock is not None:
                            state["unlocked"] = unlock
                        n -= 1

                for ph in range(2):
                    base = xb[:32768, :] if ph == 0 else xb[XB_ROWS - 32768:, :]
                    n_gather = (n_bank[ph] * BANK) // GATHER
                    for gb in range(n_gather):
                        idx_sb = iopool.tile([128, GATHER // 16], i16, tag="idx",
                                             name="idx_sb")
                        nc.sync.dma_start(
                            idx_sb[:],
                            gidx[ph][:, gb * (GATHER // 16):(gb + 1) * (GATHER // 16)])
                        # HW dma_gather caps at 512 idxs: one gather+tile per
                        # bank, sharing one batched idx load per 4096 slots.
                        # Single shared ring tag bounds outstanding gathers
                        # (SWDGE descriptor ring holds ~1024 descs).
                        rhs_tiles = []
                        for bb in range(GATHER // BANK):
                            rhs = iopool.tile([128, 1, BANK], bf16, tag="rhs",
                                              name="rhs", bufs=10)
                            nc.gpsimd.dma_gather(
                                out_ap=rhs[:],
                                in_ap=base,
                                idxs_ap=idx_sb[:, bb * (BANK // 16):(bb + 1) * (BANK // 16)],
                                num_idxs=BANK, num_idxs_reg=BANK,
                                elem_size=D_IN, transpose=True,
                                queue_num=(gb * (GATHER // BANK) + bb) % 4)
                            rhs_tiles.append(rhs)
                        if stage == "gather":
                            nc.vector.tensor_tensor(
                                out=agg[ph][:, 0, :256], in0=rhs_tiles[0][:, 0, :256],
                                in1=rhs_tiles[0][:, 0, 256:512], op=MAX)
                            continue
                        for bb in range(GATHER // BANK):
                            bank_idx = gb * (GATHER // BANK) + bb
                            bank_wins = wins[ph][bank_idx]
                            if not bank_wins:
                                continue
                            pm = mmpool.tile([128, 4, BANK], f32, tag="mm",
                                             name="pm")
                            rslice = rhs_tiles[bb][:, 0, :]
                            for h in range(4):
                                nc.tensor.matmul(
                                    out=pm[:, h, :],
                                    lhsT=wfc_sb[:, h * 128:(h + 1) * 128],
                                    rhs=rslice, start=True, stop=True)
                            # drain: n_dir chunks via DVE tensor_reduce from
                            # PSUM, rest via ACT copy + DVE bf16 tree
                            if stage == "direct":
                                for h in range(4):
                                    for (off, ng, g, col) in bank_wins:
                                        nc.vector.tensor_reduce(
                                            out=agg[ph][:, h, col:col + ng],
                                            in_=pm[:, h, off:off + ng * g]
                                                .rearrange("p (n g) -> p n g", g=g),
                                            axis=AX, op=MAX)
                                continue
                            # groupable: one full-bank window starting at 0
                            w0 = bank_wins[0]
                            groupable = (len(bank_wins) == 1 and w0[0] == 0
                                         and w0[1] * w0[2] == BANK)
                            joined = False
                            if groupable and open_group is not None:
                                st_o, nd_o, ph_o, g_o, col0_o, ng_o, nb_o, _ = \
                                    open_group
                                if (ph_o == ph and g_o == w0[2]
                                        and col0_o + nb_o * ng_o == w0[3]
                                        and nb_o < TREE_GROUP):
                                    joined = True
                            if not joined:
                                close_group()
                            if open_group is None:
                                n_dir = 1
                                drain_rot += 1
                                st = wpool.tile([128, 4, TREE_GROUP, BANK],
                                                bf16, tag="st", name="st")
                                if groupable:
                                    open_group = (st, n_dir, ph, w0[2], w0[3],
                                                  w0[1], 0, bank_idx)
                            else:
                                st = open_group[0]
                                n_dir = open_group[1]

                            for h in range(n_dir):
                                for (off, ng, g, col) in bank_wins:
                                    nc.vector.tensor_reduce(
                                        out=agg[ph][:, h, col:col + ng],
                                        in_=pm[:, h, off:off + ng * g]
                                            .rearrange("p (n g) -> p n g", g=g),
                                        axis=AX, op=MAX)

                            if groupable:
                                plane = open_group[6]
                                if n_dir < 4:
                                    nc.scalar.copy(
                                        out=st[:, n_dir:, plane, :],
                                        in_=pm[:, n_dir:, :])
                                open_group = (st, n_dir, ph, open_group[3],
                                              open_group[4], open_group[5],
                                              plane + 1, bank_idx)
                                if open_group[6] == TREE_GROUP:
                                    close_group()
                                pump(2)
                            else:
                                # irregular bank: per-window trees, queued
                                if n_dir < 4:
                                    nc.scalar.copy(out=st[:, n_dir:, 0, :],
                                                   in_=pm[:, n_dir:, :])
                                    ops = []
                                    for (off, ng, g, col) in bank_wins:
                                        v = st[:, n_dir:, 0, off:off + ng * g] \
                                            .rearrange("p k (n g) -> p k n g",
                                                       g=g)
                                        ops += _tree_ops(
                                            nc, mybir, v,
                                            agg[ph][:, n_dir:, col:col + ng],
                                            g)
                                    for op in ops:
                                        op()
                                unlock = (bank1_next_col[bank_idx]
                                          if ph == 1 else None)
                                tree_q.append((None, unlock))
                            if ph == 1 and stage == "full":
                                merge_blocks(state["unlocked"])
                                fin_pump(2)

                    close_group()
                    pump(len(tree_q))
                merge_blocks(capP)
                fin_pump(len(fin_q))

            # final data-parallel matmul over node chunks
            with tc.tile_pool(name="fin", bufs=8, space="PSUM") as finpool:
                n_chunk = capP // 128
                mb = 0
                while mb < n_chunk:
                    nb = min(8, n_chunk - mb)
                    osb = ostpool.tile([128, nb, D_OUT], f32, tag="osb",
                                       name="osb")
                    for mi in range(nb):
                        m = mb + mi
                        pm2 = finpool.tile([128, D_OUT], f32, tag="fmm",
                                           name="pm2")
                        for c in range(5):
                            lhsT = (xt_sb[:, m * 128:(m + 1) * 128] if c == 0
                                    else agg[0][:, c - 1,
                                                m * 128:(m + 1) * 128])
                            nc.tensor.matmul(
                                out=pm2[:], lhsT=lhsT,
                                rhs=wout_sb[:, c * D_OUT:(c + 1) * D_OUT],
                                start=(c == 0), stop=(c == 4))
                        nc.scalar.copy(out=osb[:, mi, :], in_=pm2[:])
                    nc.sync.dma_start(
                        outp[mb * 128:(mb + nb) * 128, :]
                            .rearrange("(c n) f -> n c f", c=nb),
                        osb[:])
                    mb += nb

    nc.finalize()
    return nc


def kernel(input_matrix, fc_w, fc_b, weights_matrix, adjacency_coo_matrix):
    global LAST_RESULTS
    from concourse.bass_utils import run_bass_kernel_spmd

    X = np.asarray(input_matrix, np.float32)
    Wfc = np.asarray(fc_w, np.float32)
    bfc = np.asarray(fc_b, np.float32)
    Wout = np.asarray(weights_matrix, np.float32)

    adj = np.asarray(adjacency_coo_matrix)
    hkey = hash(adj[:, ::1024].tobytes()) ^ hash(adj[:, -7:].tobytes())
    host = _host_cache.get(hkey)
    if host is None:
        host = _build_host_structures(adj)
        _host_cache[hkey] = host

    nc = _compiled.get(hkey)
    if nc is None:
        nc = _build_program(host)
        _compiled[hkey] = nc

    capP = host["capP"]
    Xb = X.astype(ml_dtypes.bfloat16)
    wfc_in = Wfc.astype(ml_dtypes.bfloat16)
    fcb_in = np.ascontiguousarray(bfc.reshape(4, 128).T.astype(np.float32))
    wout_in = np.ascontiguousarray(
        Wout.reshape(5, 128, D_OUT).transpose(1, 0, 2).reshape(128, 5 * D_OUT)
    ).astype(ml_dtypes.bfloat16)

    in_maps = []
    for k in range(NCORES):
        hc = host["cores"][k]
        col_node = hc["col_node"]
        xt_in = np.zeros((capP, D_IN), np.float32)
        valid = col_node >= 0
        xt_in[:len(col_node)][valid] = X[col_node[valid]]
        xt_in = np.ascontiguousarray(xt_in.T).astype(ml_dtypes.bfloat16)
        xb_core = np.zeros((XB_ROWS, D_IN), ml_dtypes.bfloat16)
        xb_core[PAD:PAD + N_NODES] = Xb
        if len(hc["pad0"]):
            xb_core[:len(hc["pad0"])] = Xb[hc["pad0"]]
        if len(hc["pad1"]):
            xb_core[PAD + N_NODES:PAD + N_NODES + len(hc["pad1"])] = Xb[hc["pad1"]]
        in_maps.append({
            "xb": xb_core,
            "gidx0": hc["gidx0"],
            "gidx1": hc["gidx1"],
            "wfc": wfc_in,
            "fcb": fcb_in,
            "xt": xt_in,
            "wout": wout_in,
        })

    res = run_bass_kernel_spmd(nc, in_maps, list(range(NCORES)))
    LAST_RESULTS = res

    out_full = np.zeros((N_NODES, D_OUT), np.float32)
    for k in range(NCORES):
        got = np.asarray(res.results[k]["out"], np.float32)
        col_node = host["cores"][k]["col_node"]
        valid = col_node >= 0
        out_full[col_node[valid]] = got[:len(col_node)][valid]
    return out_full
